# revision 11
# baseline (speedup 1.0000x reference)
"""Multi-head attention Trainium2 kernel (8 NeuronCores).

Sharding: data-parallel over batch (4 pairs of cores) x tensor-parallel over
heads (2-way split within each pair). Core c handles batch c//2 and heads
(c%2)*8 .. (c%2)*8+8.

Key structural choices vs. a straightforward flash-style kernel:
 - ctx is computed TRANSPOSED: out[q,65] = e[k, q-slice].T @ vav[k, 65]
   per (q-tile, k-block). This streams 65 columns per matmul instead of
   512, halving PE time for the ctx accumulation, keeps the ones-column
   trick (col 64 accumulates softmax row sums), and makes the softmax
   divisor a per-partition scalar (cheap tensor_scalar, no partition
   broadcast). A tiny PE transpose (identity matmul) flips the
   normalized [q, d] ctx back to [d, q] for the output projection.
 - Output projection is split by OUTPUT COLUMNS across each core pair:
   the pair AllGathers normalized ctx pieces (bf16) per (q-chunk,
   head-pair), and each core computes y[:, mycols] = relu(ctx_all @
   Wo[:, mycols] + bo[mycols]) for ALL rows. The host concatenates
   column halves. No ReduceScatter of fp32 partials, no post-collective
   bias/relu pass, no rank-dependent indexing in the program.
 - reference subtracts the row max inside exp and adds EPS=1e-7 to the
   denominator. Scores are >= 0 and s/8 <= ~6.3 here, so exp(s/8 - 1)
   is exact enough: the constant cancels in the softmax ratio and the
   denominator is >= e^-1 >> EPS.
 - causal mask is applied as an additive -3200 on pre-exp scores of the
   diagonal 128x128 squares (exp -> exactly 0), replacing the
   reference's post-exp multiplicative mask.
"""

import numpy as np
import ml_dtypes

B, S, D, H = 4, 2048, 1024, 16
HD = 64          # head dim
HC = 8           # heads per core
DC = HC * HD     # 512 head-dims per core
NCORES = 8

_cache = {}


def _build():
    import concourse.bass as bass
    import concourse.mybir as mybir
    import concourse.tile as tile
    from concourse import bacc
    from concourse.masks import make_upper_triangular, make_identity

    f32 = mybir.dt.float32
    bf16 = mybir.dt.bfloat16
    AF = mybir.ActivationFunctionType
    ALU = mybir.AluOpType

    nc = bacc.Bacc("TRN2", target_bir_lowering=False, debug=False,
                   num_devices=NCORES)

    xT_d = nc.dram_tensor("xT", [D, S], bf16, kind="ExternalInput")
    wq_d = nc.dram_tensor("wq", [D, DC], bf16, kind="ExternalInput")
    wk_d = nc.dram_tensor("wk", [D, DC], bf16, kind="ExternalInput")
    wv_d = nc.dram_tensor("wv", [D, DC], bf16, kind="ExternalInput")
    wo_d = nc.dram_tensor("wo", [D, 512], bf16, kind="ExternalInput")
    bq_d = nc.dram_tensor("bq", [128, 4], f32, kind="ExternalInput")
    bk_d = nc.dram_tensor("bk", [128, 4], f32, kind="ExternalInput")
    bvb_d = nc.dram_tensor("bvb", [128, DC], f32, kind="ExternalInput")
    bob_d = nc.dram_tensor("bob", [128, 512], f32, kind="ExternalInput")
    y_d = nc.dram_tensor("y", [S, 512], f32, kind="ExternalOutput")

    # pair-exchange buffers: 16 pieces, one per (q-chunk j, head-pair p)
    cin_d = nc.dram_tensor("cin", [16, 128, 512], bf16, kind="Internal")
    cout_d = nc.dram_tensor("cout", [16, 256, 512], bf16, kind="Internal")

    NCH = D // 128          # 8 contraction chunks for projections
    EXPB = -1.0             # exp(s/8 - 1); cancels in softmax ratio
    GROUPS = [[0, 1], [2, 3], [4, 5], [6, 7]]

    lowp = nc.allow_low_precision("bf16 matmul inputs")
    lowp.__enter__()
    with tile.TileContext(nc) as tc:
        with (
            tc.tile_pool(name="const", bufs=1) as cp,
            tc.tile_pool(name="xt", bufs=1) as xp,
            tc.tile_pool(name="proj", bufs=1) as pp,
            tc.tile_pool(name="ework", bufs=1) as ep,
            tc.tile_pool(name="small", bufs=1) as sp,
            tc.tile_pool(name="gath", bufs=1) as gp,
            tc.tile_pool(name="evac", bufs=1) as vp,
            tc.tile_pool(name="ps", bufs=2, space="PSUM") as psp,
            tc.tile_pool(name="ctxps", bufs=1, space="PSUM") as cxp,
        ):
            # ---- constants ----
            trineg = cp.tile([128, 128], f32, name="trineg", tag="trineg")
            make_upper_triangular(nc, trineg[:], val=3200.0, diag=True)
            nc.vector.tensor_scalar(trineg[:], trineg[:], -3200.0, None,
                                    ALU.add)
            ident = cp.tile([128, 128], bf16, name="ident", tag="ident")
            make_identity(nc, ident[:])
            ones_f = cp.tile([128, 64], bf16, name="ones_f", tag="ones_f")
            nc.vector.memset(ones_f[:], 1.0)
            expb_t = cp.tile([128, 1], f32, name="expb", tag="expb")
            nc.vector.memset(expb_t[:], EXPB)
            bq_t = cp.tile([128, 4], f32, name="bq", tag="bq")
            nc.gpsimd.dma_start(bq_t[:], bq_d[:])
            bk_t = cp.tile([128, 4], f32, name="bk", tag="bk")
            nc.gpsimd.dma_start(bk_t[:], bk_d[:])
            bvb_t = cp.tile([128, DC], f32, name="bvb", tag="bvb")
            nc.gpsimd.dma_start(bvb_t[:], bvb_d[:])
            bob_t = cp.tile([128, 512], f32, name="bob", tag="bob")
            nc.gpsimd.dma_start(bob_t[:], bob_d[:])

            # ---- x^T resident + wq, finely interleaved so matmuls start
            # as soon as (wq[c], xt[c][0]) pairs land ----
            xt = [[None] * 4 for _ in range(NCH)]
            wq_pre = []
            for c in range(NCH):
                wt = pp.tile([128, DC], bf16, name=f"wq{c}", tag=f"w{c}")
                (nc.scalar if c % 2 == 0 else nc.sync).dma_start(
                    wt[:], wq_d[c * 128:(c + 1) * 128, :])
                wq_pre.append(wt)
                t = xp.tile([128, 512], bf16, name=f"xt{c}_0", tag=f"xt{c}_0")
                nc.gpsimd.dma_start(t[:], xT_d[c * 128:(c + 1) * 128, 0:512])
                xt[c][0] = t
            for sg in range(1, 4):
                for c in range(NCH):
                    t = xp.tile([128, 512], bf16, name=f"xt{c}_{sg}",
                                tag=f"xt{c}_{sg}")
                    eng = (nc.sync, nc.scalar, nc.gpsimd)[c % 3]
                    t_src = xT_d[c * 128:(c + 1) * 128,
                                 sg * 512:(sg + 1) * 512]
                    eng.dma_start(t[:], t_src)
                    xt[c][sg] = t

            # ---- q^T and k^T projections: [DC, S] as 4 tiles [128, S] ----
            # tile t holds local heads 2t (partitions 0:64) and 2t+1 (64:128)
            qT, kT = [], []
            for (w_d, bias_t, out_list, nm) in (
                (wq_d, bq_t, qT, "q"), (wk_d, bk_t, kT, "k"),
            ):
                if nm == "q":
                    wch = wq_pre
                else:
                    wch = []
                    for c in range(NCH):
                        wt = pp.tile([128, DC], bf16, name=f"w{nm}{c}",
                                     tag=f"w{c}")
                        (nc.scalar if c % 2 == 0 else nc.sync).dma_start(
                            wt[:], w_d[c * 128:(c + 1) * 128, :])
                        wch.append(wt)
                for seg in range(4):
                    for t in range(4):
                        if seg == 0:
                            out = pp.tile([128, S], bf16, name=f"{nm}T{t}",
                                          tag=f"{nm}T{t}")
                            out_list.append(out)
                        out = out_list[t]
                        ps = psp.tile([128, 512], f32, name="ps",
                                      tag="big", bufs=2)
                        for c in range(NCH):
                            nc.tensor.matmul(
                                ps[:],
                                wch[c][:, t * 128:(t + 1) * 128],
                                xt[c][seg][:],
                                start=(c == 0), stop=(c == NCH - 1),
                            )
                        nc.vector.tensor_scalar(
                            out[:, seg * 512:(seg + 1) * 512], ps[:],
                            bias_t[:, t:t + 1], 0.0,
                            ALU.add, ALU.max,
                        )

            # ---- v projection into [128, HC, 65] per 128-token block ----
            # cols h,0:64 = relu(x@wv+bv) of local head h; col h,64 = 1.0
            wvch = []
            for c in range(NCH):
                wt = pp.tile([128, DC], bf16, name=f"wv{c}", tag=f"w{c}")
                (nc.scalar if c % 2 == 0 else nc.sync).dma_start(
                    wt[:], wv_d[c * 128:(c + 1) * 128, :])
                wvch.append(wt)
            vav = []
            for st in range(16):
                va = pp.tile([128, HC, 65], bf16, name=f"va{st}",
                             tag=f"va{st}")
                vav.append(va)
                ps = psp.tile([128, 512], f32, name="ps", tag="big", bufs=2)
                for c in range(NCH):
                    nc.tensor.matmul(
                        ps[:],
                        xt[c][st // 4][:, (st % 4) * 128:(st % 4) * 128 + 128],
                        wvch[c][:],
                        start=(c == 0), stop=(c == NCH - 1),
                    )
                nc.vector.tensor_add(ps[:], ps[:], bvb_t[:])
                nc.vector.tensor_scalar(
                    va[:, :, 0:64],
                    ps[:].rearrange("p (h d) -> p h d", h=HC),
                    0.0, None, ALU.max,
                )
                nc.vector.tensor_copy(
                    va[:, :, 64:65],
                    ones_f[:, 0:8].rearrange("p (h o) -> p h o", o=1))

            # ---- wo tiles (needed from o-proj(0) on) ----
            woch = []
            for c in range(NCH):
                wt = pp.tile([128, 512], bf16, name=f"wo{c}", tag=f"wo{c}")
                (nc.gpsimd if c % 2 == 0 else nc.scalar).dma_start(
                    wt[:], wo_d[c * 128:(c + 1) * 128, :])
                woch.append(wt)

            glist = {}

            def attention(j):
                nblk = 4 * j + 4
                for p in range(4):
                    # ctx accumulators, one per head: [q 128, qt 4, 65]
                    cx = [cxp.tile([128, 4, 65], f32, name=f"cx{hh}",
                                   tag=f"cx{hh}", bufs=1) for hh in (0, 1)]
                    for kb in range(nblk):
                        dlt = kb * 128 - j * 512
                        qoff = max(dlt, 0)
                        w = 512 - qoff
                        qlo = j * 512 + qoff
                        sc = psp.tile([128, 1024], f32, name="sc",
                                      tag="sc", bufs=2)
                        e2 = ep.tile([128, 1024], bf16, name="e", tag="e",
                                     bufs=5)
                        for hh in (0, 1):
                            plo = hh * 64
                            nc.tensor.matmul(
                                sc[:, hh * 512 + qoff:hh * 512 + 512],
                                kT[p][plo:plo + 64,
                                      kb * 128:(kb + 1) * 128],
                                qT[p][plo:plo + 64, qlo:qlo + w],
                                start=True, stop=True,
                                tile_position=(plo, 0),
                            )
                        if dlt >= 0:
                            # additive causal mask on the diag square
                            nc.vector.tensor_add(
                                sc[:, qoff:qoff + 128],
                                sc[:, qoff:qoff + 128], trineg[:])
                            nc.vector.tensor_add(
                                sc[:, 512 + qoff:512 + qoff + 128],
                                sc[:, 512 + qoff:512 + qoff + 128],
                                trineg[:])
                        if w == 512:
                            nc.scalar.activation(
                                e2[:], sc[:],
                                AF.Exp, bias=expb_t[:], scale=0.125,
                            )
                        else:
                            for hh in (0, 1):
                                nc.scalar.activation(
                                    e2[:, hh * 512 + qoff:hh * 512 + 512],
                                    sc[:, hh * 512 + qoff:hh * 512 + 512],
                                    AF.Exp, bias=expb_t[:], scale=0.125,
                                )
                        # transposed ctx: out [q, 65] streams only 65 cols
                        for qt in range(4):
                            if 128 * qt < dlt:
                                continue
                            for hh in (0, 1):
                                # one start/stop per PSUM bank: start=True
                                # zeroes the whole 2KB bank (ZERO_REGION),
                                # so only the bank's first matmul starts
                                # and only its last stops; other regions
                                # accumulate onto pending-zero bytes.
                                nc.tensor.matmul(
                                    cx[hh][:, qt, :],
                                    e2[:, hh * 512 + qt * 128:
                                        hh * 512 + (qt + 1) * 128],
                                    vav[kb][:, 2 * p + hh, :],
                                    start=(kb == 0 and qt == 0),
                                    stop=(kb == nblk - 1 and qt == 3),
                                    skip_group_check=True,
                                )
                    # normalize rows by col 64 (per-partition scalar), then
                    # PE-transpose [q,d] -> [d,q] and stage for exchange
                    stq = sp.tile([128, 4, 128], bf16, name="stq", tag="stq",
                                  bufs=2)
                    for qt in range(4):
                        for hh in (0, 1):
                            rc = sp.tile([128, 1], f32, name="rc", tag="rc",
                                         bufs=4)
                            nc.vector.reciprocal_approx_fast(
                                rc[:], cx[hh][:, qt, 64:65])
                            nc.vector.tensor_scalar(
                                stq[:, qt, hh * 64:hh * 64 + 64],
                                cx[hh][:, qt, 0:64],
                                rc[:], None, ALU.mult,
                            )
                    tpp = psp.tile([128, 512], bf16, name="tp", tag="big",
                                   bufs=2)
                    for qt in range(4):
                        # start only on the bank's first transpose (see
                        # ZERO_REGION note above)
                        nc.tensor.matmul(
                            tpp[:, qt * 128:(qt + 1) * 128],
                            stq[:, qt, :], ident[:],
                            is_transpose=True,
                            start=(qt == 0), stop=(qt == 3),
                            skip_group_check=True,
                        )
                    stgT = sp.tile([128, 512], bf16, name="stgT", tag="stgT",
                                   bufs=3)
                    nc.vector.tensor_copy(stgT[:], tpp[:])
                    idx = j * 4 + p
                    nc.sync.dma_start(cin_d[idx], stgT[:])
                    nc.gpsimd.collective_compute(
                        "AllGather",
                        mybir.AluOpType.bypass,
                        replica_groups=GROUPS,
                        ins=[cin_d[idx].opt()],
                        outs=[cout_d[idx].opt()],
                    )
                    gA = gp.tile([128, 512], bf16, name=f"g{p}a",
                                 tag=f"g{p}a", bufs=2)
                    gB = gp.tile([128, 512], bf16, name=f"g{p}b",
                                 tag=f"g{p}b", bufs=2)
                    nc.scalar.dma_start(gA[:], cout_d[idx, 0:128, :])
                    nc.sync.dma_start(gB[:], cout_d[idx, 128:256, :])
                    glist.setdefault(j, []).append((gA, gB))

            def oproj(j):
                # contraction order: [gA0..gA3, gB0..gB3] == natural Wo rows
                chunks = [g[0] for g in glist[j]] + [g[1] for g in glist[j]]
                for qt in range(4):
                    op = psp.tile([128, 512], f32, name="op", tag="big",
                                  bufs=2)
                    for c in range(NCH):
                        nc.tensor.matmul(
                            op[:],
                            chunks[c][:, qt * 128:(qt + 1) * 128],
                            woch[c][:],
                            start=(c == 0), stop=(c == NCH - 1),
                        )
                    ys = vp.tile([128, 512], f32, name="ys", tag="ys",
                                 bufs=3)
                    nc.vector.tensor_add(ys[:], op[:], bob_t[:])
                    nc.gpsimd.tensor_scalar(ys[:], ys[:], 0.0, None, ALU.max)
                    nc.sync.dma_start(
                        y_d[j * 512 + qt * 128:j * 512 + (qt + 1) * 128, :],
                        ys[:])

            attention(0)
            attention(1)
            oproj(0)
            attention(2)
            oproj(1)
            attention(3)
            oproj(2)
            oproj(3)

    lowp.__exit__(None, None, None)
    nc.compile()
    return nc


def _get_nc():
    if "nc" not in _cache:
        _cache["nc"] = _build()
    return _cache["nc"]


def kernel(x, Wq, bq, Wk, bk, Wv, bv, Wo, bo, trace=False):
    from concourse.bass_utils import run_bass_kernel_spmd

    x = np.asarray(x, np.float32)
    Wq, bq = np.asarray(Wq, np.float32), np.asarray(bq, np.float32)
    Wk, bk = np.asarray(Wk, np.float32), np.asarray(bk, np.float32)
    Wv, bv = np.asarray(Wv, np.float32), np.asarray(bv, np.float32)
    Wo, bo = np.asarray(Wo, np.float32), np.asarray(bo, np.float32)

    nc = _get_nc()
    in_maps = []
    for c in range(NCORES):
        b, hh = c // 2, c % 2
        sl = slice(hh * DC, (hh + 1) * DC)
        cols = slice(hh * 512, (hh + 1) * 512)
        in_maps.append({
            "xT": np.ascontiguousarray(x[b].T).astype(ml_dtypes.bfloat16),
            "wq": np.ascontiguousarray(Wq[:, sl]).astype(ml_dtypes.bfloat16),
            "wk": np.ascontiguousarray(Wk[:, sl]).astype(ml_dtypes.bfloat16),
            "wv": np.ascontiguousarray(Wv[:, sl]).astype(ml_dtypes.bfloat16),
            "wo": np.ascontiguousarray(Wo[:, cols]).astype(ml_dtypes.bfloat16),
            "bq": np.ascontiguousarray(bq[sl].reshape(4, 128).T),
            "bk": np.ascontiguousarray(bk[sl].reshape(4, 128).T),
            "bvb": np.ascontiguousarray(
                np.broadcast_to(bv[sl], (128, DC))),
            "bob": np.ascontiguousarray(np.broadcast_to(bo[cols], (128, 512))),
        })

    res = run_bass_kernel_spmd(nc, in_maps, core_ids=list(range(NCORES)),
                               trace=trace)
    _cache["last_result"] = res

    y = np.empty((B, S, D), np.float32)
    for c in range(NCORES):
        b, hh = c // 2, c % 2
        y[b, :, hh * 512:(hh + 1) * 512] = res.results[c]["y"]
    return y


# revision 21
# speedup vs baseline: 1.0689x; 1.0689x over previous
"""Multi-head attention Trainium2 kernel (8 NeuronCores).

Sharding: data-parallel over batch (4 pairs of cores) x tensor-parallel over
heads (2-way split within each pair). Core c handles batch c//2 and heads
(c%2)*8 .. (c%2)*8+8.

Key structural choices vs. a straightforward flash-style kernel:
 - ctx accumulates as [65, q]: lhsT = vav [k,65] (64 v-dims + a ones
   column whose output row accumulates the softmax row sums for free),
   rhs = e [k, q] streaming the q window. (A transposed variant that
   streams only 65 columns was tried and is NOT faster: the e matrix
   must cross the PE either as weights or as stream, and 128-column
   weight loads exceed the 65-column streams, so it is load-bound at
   the same total cycles with 3x the instruction overhead.)
 - Output projection is split by OUTPUT COLUMNS across each core pair:
   the pair AllGathers normalized ctx pieces (bf16) per (q-chunk,
   head-pair), and each core computes y[:, mycols] = relu(ctx_all @
   Wo[:, mycols] + bo[mycols]) for ALL rows. The host concatenates
   column halves. No ReduceScatter of fp32 partials, no post-collective
   bias/relu pass, no rank-dependent indexing in the program.
 - reference subtracts the row max inside exp and adds EPS=1e-7 to the
   denominator. Scores are >= 0 and s/8 <= ~6.3 here, so exp(s/8 - 1)
   is exact enough: the constant cancels in the softmax ratio and the
   denominator is >= e^-1 >> EPS.
 - causal mask is applied as an additive -3200 on pre-exp scores of the
   diagonal 128x128 squares (exp -> exactly 0), replacing the
   reference's post-exp multiplicative mask.
"""

import numpy as np
import ml_dtypes

B, S, D, H = 4, 2048, 1024, 16
HD = 64          # head dim
HC = 8           # heads per core
DC = HC * HD     # 512 head-dims per core
NCORES = 8

_cache = {}


def _build():
    import concourse.bass as bass
    import concourse.mybir as mybir
    import concourse.tile as tile
    from concourse import bacc
    from concourse.masks import make_upper_triangular

    f32 = mybir.dt.float32
    bf16 = mybir.dt.bfloat16
    AF = mybir.ActivationFunctionType
    ALU = mybir.AluOpType

    nc = bacc.Bacc("TRN2", target_bir_lowering=False, debug=False,
                   num_devices=NCORES)

    xT_d = nc.dram_tensor("xT", [D, S], bf16, kind="ExternalInput")
    wq_d = nc.dram_tensor("wq", [D, DC], bf16, kind="ExternalInput")
    wk_d = nc.dram_tensor("wk", [D, DC], bf16, kind="ExternalInput")
    wv_d = nc.dram_tensor("wv", [D, DC], bf16, kind="ExternalInput")
    wo_d = nc.dram_tensor("wo", [D, 512], bf16, kind="ExternalInput")
    bq_d = nc.dram_tensor("bq", [128, 4], f32, kind="ExternalInput")
    bk_d = nc.dram_tensor("bk", [128, 4], f32, kind="ExternalInput")
    bvb_d = nc.dram_tensor("bvb", [128, DC], f32, kind="ExternalInput")
    bob_d = nc.dram_tensor("bob", [128, 512], f32, kind="ExternalInput")
    y_d = nc.dram_tensor("y", [S, 512], f32, kind="ExternalOutput")

    NCH = D // 128          # 8 contraction chunks for projections
    EXPB = -1.0             # exp(s/8 - 1); cancels in softmax ratio
    GROUPS = [[0, 1], [2, 3], [4, 5], [6, 7]]

    lowp = nc.allow_low_precision("bf16 matmul inputs")
    lowp.__enter__()
    with tile.TileContext(nc) as tc:
        with (
            tc.tile_pool(name="const", bufs=1) as cp,
            tc.tile_pool(name="xt", bufs=1) as xp,
            tc.tile_pool(name="proj", bufs=1) as pp,
            tc.tile_pool(name="ework", bufs=1) as ep,
            tc.tile_pool(name="small", bufs=1) as sp,
            tc.tile_pool(name="gath", bufs=1) as gp,
            tc.tile_pool(name="evac", bufs=1) as vp,
            tc.tile_pool(name="ps", bufs=2, space="PSUM") as psp,
            tc.tile_pool(name="ctxps", bufs=1, space="PSUM") as cxp,
            tc.tile_pool(name="dram", bufs=1, space="DRAM") as dp,
        ):
            # pair-exchange buffers (pool tiles so Tile tracks the
            # DMA-write -> AllGather -> DMA-read dependency chain);
            # 16 pieces, one per (q-chunk j, head-pair p)
            cin_d = dp.tile([16, 128, 512], bf16, name="cin", tag="cin")
            cout_d = dp.tile([16, 256, 512], bf16, name="cout", tag="cout")
            # ---- constants ----
            trineg = cp.tile([128, 128], f32, name="trineg", tag="trineg")
            make_upper_triangular(nc, trineg[:], val=3200.0, diag=True)
            nc.vector.tensor_scalar(trineg[:], trineg[:], -3200.0, None,
                                    ALU.add)
            ones_f = cp.tile([128, 64], bf16, name="ones_f", tag="ones_f")
            nc.vector.memset(ones_f[:], 1.0)
            expb_t = cp.tile([128, 1], f32, name="expb", tag="expb")
            nc.vector.memset(expb_t[:], EXPB)
            bq_t = cp.tile([128, 4], f32, name="bq", tag="bq")
            nc.gpsimd.dma_start(bq_t[:], bq_d[:])
            bk_t = cp.tile([128, 4], f32, name="bk", tag="bk")
            nc.gpsimd.dma_start(bk_t[:], bk_d[:])
            bvb_t = cp.tile([128, DC], f32, name="bvb", tag="bvb")
            nc.gpsimd.dma_start(bvb_t[:], bvb_d[:])
            bob_t = cp.tile([128, 512], f32, name="bob", tag="bob")
            nc.gpsimd.dma_start(bob_t[:], bob_d[:])

            # ---- x^T resident + wq, finely interleaved so matmuls start
            # as soon as (wq[c], xt[c][0]) pairs land ----
            xt = [[None] * 4 for _ in range(NCH)]
            wq_pre = []
            for c in range(NCH):
                wt = pp.tile([128, DC], bf16, name=f"wq{c}", tag=f"w{c}")
                (nc.scalar if c % 2 == 0 else nc.sync).dma_start(
                    wt[:], wq_d[c * 128:(c + 1) * 128, :])
                wq_pre.append(wt)
                t = xp.tile([128, 512], bf16, name=f"xt{c}_0", tag=f"xt{c}_0")
                nc.gpsimd.dma_start(t[:], xT_d[c * 128:(c + 1) * 128, 0:512])
                xt[c][0] = t
            for sg in range(1, 4):
                for c in range(NCH):
                    t = xp.tile([128, 512], bf16, name=f"xt{c}_{sg}",
                                tag=f"xt{c}_{sg}")
                    eng = (nc.sync, nc.scalar, nc.gpsimd)[c % 3]
                    t_src = xT_d[c * 128:(c + 1) * 128,
                                 sg * 512:(sg + 1) * 512]
                    eng.dma_start(t[:], t_src)
                    xt[c][sg] = t

            # ---- q^T and k^T projections: [DC, S] as 4 tiles [128, S] ----
            # tile t holds local heads 2t (partitions 0:64) and 2t+1 (64:128)
            qT, kT = [], []
            for (w_d, bias_t, out_list, nm) in (
                (wq_d, bq_t, qT, "q"), (wk_d, bk_t, kT, "k"),
            ):
                if nm == "q":
                    wch = wq_pre
                else:
                    wch = []
                    for c in range(NCH):
                        wt = pp.tile([128, DC], bf16, name=f"w{nm}{c}",
                                     tag=f"w{c}")
                        (nc.scalar if c % 2 == 0 else nc.sync).dma_start(
                            wt[:], w_d[c * 128:(c + 1) * 128, :])
                        wch.append(wt)
                for seg in range(4):
                    for t in range(4):
                        if seg == 0:
                            out = pp.tile([128, S], bf16, name=f"{nm}T{t}",
                                          tag=f"{nm}T{t}")
                            out_list.append(out)
                        out = out_list[t]
                        ps = psp.tile([128, 512], f32, name="ps",
                                      tag="big", bufs=2)
                        for c in range(NCH):
                            nc.tensor.matmul(
                                ps[:],
                                wch[c][:, t * 128:(t + 1) * 128],
                                xt[c][seg][:],
                                start=(c == 0), stop=(c == NCH - 1),
                            )
                        nc.vector.tensor_scalar(
                            out[:, seg * 512:(seg + 1) * 512], ps[:],
                            bias_t[:, t:t + 1], 0.0,
                            ALU.add, ALU.max,
                        )

            # ---- v projection into [128, HC, 65] per 128-token block ----
            # cols h,0:64 = relu(x@wv+bv) of local head h; col h,64 = 1.0
            wvch = []
            for c in range(NCH):
                wt = pp.tile([128, DC], bf16, name=f"wv{c}", tag=f"w{c}")
                (nc.scalar if c % 2 == 0 else nc.sync).dma_start(
                    wt[:], wv_d[c * 128:(c + 1) * 128, :])
                wvch.append(wt)
            vav = []
            for st in range(16):
                va = pp.tile([128, HC, 65], bf16, name=f"va{st}",
                             tag=f"va{st}")
                vav.append(va)
                ps = psp.tile([128, 512], f32, name="ps", tag="big", bufs=2)
                for c in range(NCH):
                    nc.tensor.matmul(
                        ps[:],
                        xt[c][st // 4][:, (st % 4) * 128:(st % 4) * 128 + 128],
                        wvch[c][:],
                        start=(c == 0), stop=(c == NCH - 1),
                    )
                nc.vector.tensor_add(ps[:], ps[:], bvb_t[:])
                nc.vector.tensor_scalar(
                    va[:, :, 0:64],
                    ps[:].rearrange("p (h d) -> p h d", h=HC),
                    0.0, None, ALU.max,
                )
                nc.vector.tensor_copy(
                    va[:, :, 64:65],
                    ones_f[:, 0:8].rearrange("p (h o) -> p h o", o=1))

            # ---- wo tiles (needed from o-proj(0) on) ----
            woch = []
            for c in range(NCH):
                wt = pp.tile([128, 512], bf16, name=f"wo{c}", tag=f"wo{c}")
                (nc.gpsimd if c % 2 == 0 else nc.scalar).dma_start(
                    wt[:], wo_d[c * 128:(c + 1) * 128, :])
                woch.append(wt)

            glist = {}

            def attention(j):
                nblk = 4 * j + 4
                for p in range(4):
                    # ctx accumulators [d 64 + rowsum, q 512], one per head
                    ctxA = cxp.tile([65, 512], f32, name="cx", tag="cx",
                                    bufs=2)
                    ctxB = cxp.tile([65, 512], f32, name="cx", tag="cx",
                                    bufs=2)
                    for kb in range(nblk):
                        dlt = kb * 128 - j * 512
                        qoff = max(dlt, 0)
                        w = 512 - qoff
                        qlo = j * 512 + qoff
                        sc = psp.tile([128, 1024], f32, name="sc",
                                      tag="sc", bufs=2)
                        e2 = ep.tile([128, 1024], bf16, name="e", tag="e",
                                     bufs=5)
                        for hh in (0, 1):
                            plo = hh * 64
                            nc.tensor.matmul(
                                sc[:, hh * 512 + qoff:hh * 512 + 512],
                                kT[p][plo:plo + 64,
                                      kb * 128:(kb + 1) * 128],
                                qT[p][plo:plo + 64, qlo:qlo + w],
                                start=True, stop=True,
                                tile_position=(plo, 0),
                            )
                        if dlt >= 0:
                            # additive causal mask on the diag square
                            nc.vector.tensor_add(
                                sc[:, qoff:qoff + 128],
                                sc[:, qoff:qoff + 128], trineg[:])
                            nc.vector.tensor_add(
                                sc[:, 512 + qoff:512 + qoff + 128],
                                sc[:, 512 + qoff:512 + qoff + 128],
                                trineg[:])
                        if w == 512:
                            nc.scalar.activation(
                                e2[:], sc[:],
                                AF.Exp, bias=expb_t[:], scale=0.125,
                            )
                        else:
                            for hh in (0, 1):
                                nc.scalar.activation(
                                    e2[:, hh * 512 + qoff:hh * 512 + 512],
                                    sc[:, hh * 512 + qoff:hh * 512 + 512],
                                    AF.Exp, bias=expb_t[:], scale=0.125,
                                )
                        for (hh, ctx) in ((0, ctxA), (1, ctxB)):
                            nc.tensor.matmul(
                                ctx[:, qoff:qoff + w],
                                vav[kb][:, 2 * p + hh, :],
                                e2[:, hh * 512 + qoff:hh * 512 + 512],
                                start=(kb == 0), stop=(kb == nblk - 1),
                                skip_group_check=True,
                            )
                    # normalize rows 0:64 by the ones-row sums (row 64) and
                    # assemble the bf16 exchange piece [128 d, 512 q]
                    stgT = sp.tile([128, 512], bf16, name="stgT", tag="stgT",
                                   bufs=3)
                    for (hh, ctx) in ((0, ctxA), (1, ctxB)):
                        rho = sp.tile([1, 512], f32, name="rho", tag="rho",
                                      bufs=4)
                        nc.vector.tensor_copy(rho[:], ctx[64:65, :])
                        rc = sp.tile([1, 512], f32, name="rc", tag="rc",
                                     bufs=4)
                        nc.vector.reciprocal_approx_fast(rc[:], rho[:])
                        rcb = sp.tile([64, 512], f32, name="rcb", tag="rcb",
                                      bufs=4)
                        nc.gpsimd.partition_broadcast(rcb[:], rc[:])
                        nc.vector.tensor_mul(
                            stgT[hh * 64:hh * 64 + 64, :], ctx[0:64, :],
                            rcb[:])
                    idx = j * 4 + p
                    nc.sync.dma_start(cin_d[idx], stgT[:])
                    nc.gpsimd.collective_compute(
                        "AllGather",
                        mybir.AluOpType.bypass,
                        replica_groups=GROUPS,
                        ins=[cin_d[idx].opt()],
                        outs=[cout_d[idx].opt()],
                    )
                    gA = gp.tile([128, 512], bf16, name=f"g{p}a",
                                 tag=f"g{p}a", bufs=2)
                    gB = gp.tile([128, 512], bf16, name=f"g{p}b",
                                 tag=f"g{p}b", bufs=2)
                    # readbacks go on the gpsimd queue (the collective's
                    # own): scalar/sync-queue reads of a collective output
                    # raced ahead of CC completion on HW
                    nc.gpsimd.dma_start(gA[:], cout_d[idx, 0:128, :])
                    nc.gpsimd.dma_start(gB[:], cout_d[idx, 128:256, :])
                    glist.setdefault(j, []).append((gA, gB))

            def oproj(j):
                # contraction order: [gA0..gA3, gB0..gB3] == natural Wo rows
                chunks = [g[0] for g in glist[j]] + [g[1] for g in glist[j]]
                for qt in range(4):
                    op = psp.tile([128, 512], f32, name="op", tag="big",
                                  bufs=2)
                    for c in range(NCH):
                        nc.tensor.matmul(
                            op[:],
                            chunks[c][:, qt * 128:(qt + 1) * 128],
                            woch[c][:],
                            start=(c == 0), stop=(c == NCH - 1),
                        )
                    ys = vp.tile([128, 512], f32, name="ys", tag="ys",
                                 bufs=3)
                    nc.vector.tensor_add(ys[:], op[:], bob_t[:])
                    nc.vector.tensor_scalar(ys[:], ys[:], 0.0, None, ALU.max)
                    nc.sync.dma_start(
                        y_d[j * 512 + qt * 128:j * 512 + (qt + 1) * 128, :],
                        ys[:])

            attention(0)
            attention(1)
            oproj(0)
            attention(2)
            oproj(1)
            attention(3)
            oproj(2)
            oproj(3)

    lowp.__exit__(None, None, None)
    nc.compile()
    return nc


def _get_nc():
    if "nc" not in _cache:
        _cache["nc"] = _build()
    return _cache["nc"]


def kernel(x, Wq, bq, Wk, bk, Wv, bv, Wo, bo, trace=False):
    from concourse.bass_utils import run_bass_kernel_spmd

    x = np.asarray(x, np.float32)
    Wq, bq = np.asarray(Wq, np.float32), np.asarray(bq, np.float32)
    Wk, bk = np.asarray(Wk, np.float32), np.asarray(bk, np.float32)
    Wv, bv = np.asarray(Wv, np.float32), np.asarray(bv, np.float32)
    Wo, bo = np.asarray(Wo, np.float32), np.asarray(bo, np.float32)

    nc = _get_nc()
    in_maps = []
    for c in range(NCORES):
        b, hh = c // 2, c % 2
        sl = slice(hh * DC, (hh + 1) * DC)
        cols = slice(hh * 512, (hh + 1) * 512)
        in_maps.append({
            "xT": np.ascontiguousarray(x[b].T).astype(ml_dtypes.bfloat16),
            "wq": np.ascontiguousarray(Wq[:, sl]).astype(ml_dtypes.bfloat16),
            "wk": np.ascontiguousarray(Wk[:, sl]).astype(ml_dtypes.bfloat16),
            "wv": np.ascontiguousarray(Wv[:, sl]).astype(ml_dtypes.bfloat16),
            "wo": np.ascontiguousarray(Wo[:, cols]).astype(ml_dtypes.bfloat16),
            "bq": np.ascontiguousarray(bq[sl].reshape(4, 128).T),
            "bk": np.ascontiguousarray(bk[sl].reshape(4, 128).T),
            "bvb": np.ascontiguousarray(
                np.broadcast_to(bv[sl], (128, DC))),
            "bob": np.ascontiguousarray(np.broadcast_to(bo[cols], (128, 512))),
        })

    res = run_bass_kernel_spmd(nc, in_maps, core_ids=list(range(NCORES)),
                               trace=trace)
    _cache["last_result"] = res

    y = np.empty((B, S, D), np.float32)
    for c in range(NCORES):
        b, hh = c // 2, c % 2
        y[b, :, hh * 512:(hh + 1) * 512] = res.results[c]["y"]
    return y


# revision 27
# speedup vs baseline: 1.2736x; 1.1915x over previous
"""Multi-head attention Trainium2 kernel (8 NeuronCores).

Sharding: data-parallel over batch (4 pairs of cores) x tensor-parallel over
heads (2-way split within each pair). Core c handles batch c//2 and heads
(c%2)*8 .. (c%2)*8+8.

Key structural choices vs. a straightforward flash-style kernel:
 - ctx accumulates as [65, q]: lhsT = vav [k,65] (64 v-dims + a ones
   column whose output row accumulates the softmax row sums for free),
   rhs = e [k, q] streaming the q window. (A transposed variant that
   streams only 65 columns was tried and is NOT faster: the e matrix
   must cross the PE either as weights or as stream, and 128-column
   weight loads exceed the 65-column streams, so it is load-bound at
   the same total cycles with 3x the instruction overhead.)
 - Output projection is split by OUTPUT COLUMNS across each core pair:
   the pair AllGathers normalized ctx pieces (bf16) per (q-chunk,
   head-pair), and each core computes y[:, mycols] = relu(ctx_all @
   Wo[:, mycols] + bo[mycols]) for ALL rows. The host concatenates
   column halves. No ReduceScatter of fp32 partials, no post-collective
   bias/relu pass, no rank-dependent indexing in the program.
 - reference subtracts the row max inside exp and adds EPS=1e-7 to the
   denominator. Scores are >= 0 and s/8 <= ~6.3 here, so exp(s/8 - 1)
   is exact enough: the constant cancels in the softmax ratio and the
   denominator is >= e^-1 >> EPS.
 - causal mask is applied as an additive -3200 on pre-exp scores of the
   diagonal 128x128 squares (exp -> exactly 0), replacing the
   reference's post-exp multiplicative mask.
"""

import numpy as np
import ml_dtypes

B, S, D, H = 4, 2048, 1024, 16
HD = 64          # head dim
HC = 8           # heads per core
DC = HC * HD     # 512 head-dims per core
NCORES = 8

_cache = {}


def _build():
    import concourse.bass as bass
    import concourse.mybir as mybir
    import concourse.tile as tile
    from concourse import bacc
    from concourse.masks import make_upper_triangular

    f32 = mybir.dt.float32
    bf16 = mybir.dt.bfloat16
    AF = mybir.ActivationFunctionType
    ALU = mybir.AluOpType

    nc = bacc.Bacc("TRN2", target_bir_lowering=False, debug=False,
                   num_devices=NCORES)

    xT_d = nc.dram_tensor("xT", [D, S], bf16, kind="ExternalInput")
    wq_d = nc.dram_tensor("wq", [D, DC], bf16, kind="ExternalInput")
    wk_d = nc.dram_tensor("wk", [D, DC], bf16, kind="ExternalInput")
    wv_d = nc.dram_tensor("wv", [D, DC], bf16, kind="ExternalInput")
    wo_d = nc.dram_tensor("wo", [D, 512], bf16, kind="ExternalInput")
    bq_d = nc.dram_tensor("bq", [128, 4], f32, kind="ExternalInput")
    bk_d = nc.dram_tensor("bk", [128, 4], f32, kind="ExternalInput")
    bvb_d = nc.dram_tensor("bvb", [128, DC], f32, kind="ExternalInput")
    bob_d = nc.dram_tensor("bob", [128, 512], f32, kind="ExternalInput")
    y_d = nc.dram_tensor("y", [S, 512], f32, kind="ExternalOutput")

    NCH = D // 128          # 8 contraction chunks for projections
    EXPB = -1.0             # exp(s/8 - 1); cancels in softmax ratio
    GROUPS = [[0, 1], [2, 3], [4, 5], [6, 7]]

    lowp = nc.allow_low_precision("bf16 matmul inputs")
    lowp.__enter__()
    with tile.TileContext(nc) as tc:
        with (
            tc.tile_pool(name="const", bufs=1) as cp,
            tc.tile_pool(name="xt", bufs=1) as xp,
            tc.tile_pool(name="proj", bufs=1) as pp,
            tc.tile_pool(name="ework", bufs=1) as ep,
            tc.tile_pool(name="small", bufs=1) as sp,
            tc.tile_pool(name="gath", bufs=1) as gp,
            tc.tile_pool(name="evac", bufs=1) as vp,
            tc.tile_pool(name="ps", bufs=2, space="PSUM") as psp,
            tc.tile_pool(name="ctxps", bufs=1, space="PSUM") as cxp,
            tc.tile_pool(name="dram", bufs=1, space="DRAM") as dp,
        ):
            # pair-exchange buffers (pool tiles so Tile tracks the
            # DMA-write -> AllGather -> DMA-read dependency chain);
            # 16 pieces, one per (q-chunk j, head-pair p)
            cin_d = dp.tile([16, 128, 512], bf16, name="cin", tag="cin")
            cout_d = dp.tile([16, 256, 512], bf16, name="cout", tag="cout")
            # ---- constants ----
            trineg = cp.tile([128, 128], f32, name="trineg", tag="trineg")
            make_upper_triangular(nc, trineg[:], val=3200.0, diag=True)
            nc.vector.tensor_scalar(trineg[:], trineg[:], -3200.0, None,
                                    ALU.add)
            ones_f = cp.tile([128, 64], bf16, name="ones_f", tag="ones_f")
            nc.vector.memset(ones_f[:], 1.0)
            expb_t = cp.tile([128, 1], f32, name="expb", tag="expb")
            nc.vector.memset(expb_t[:], EXPB)
            bq_t = cp.tile([128, 4], f32, name="bq", tag="bq")
            nc.gpsimd.dma_start(bq_t[:], bq_d[:])
            bk_t = cp.tile([128, 4], f32, name="bk", tag="bk")
            nc.gpsimd.dma_start(bk_t[:], bk_d[:])

            # ---- x^T resident + wq, finely interleaved so matmuls start
            # as soon as (wq[c], xt[c][0]) pairs land ----
            xt = [[None] * 4 for _ in range(NCH)]
            wq_pre = []
            for c in range(NCH):
                wt = pp.tile([128, DC], bf16, name=f"wq{c}", tag=f"w{c}")
                (nc.scalar if c % 2 == 0 else nc.sync).dma_start(
                    wt[:], wq_d[c * 128:(c + 1) * 128, :])
                wq_pre.append(wt)
                t = xp.tile([128, 512], bf16, name=f"xt{c}_0", tag=f"xt{c}_0")
                nc.gpsimd.dma_start(t[:], xT_d[c * 128:(c + 1) * 128, 0:512])
                xt[c][0] = t
            # biases needed later (v-proj / o-proj): off the startup path
            bvb_t = cp.tile([128, DC], f32, name="bvb", tag="bvb")
            nc.gpsimd.dma_start(bvb_t[:], bvb_d[:])
            bob_t = cp.tile([128, 512], f32, name="bob", tag="bob")
            nc.gpsimd.dma_start(bob_t[:], bob_d[:])
            for sg in range(1, 4):
                for c in range(NCH):
                    t = xp.tile([128, 512], bf16, name=f"xt{c}_{sg}",
                                tag=f"xt{c}_{sg}")
                    eng = (nc.sync, nc.scalar, nc.gpsimd)[c % 3]
                    t_src = xT_d[c * 128:(c + 1) * 128,
                                 sg * 512:(sg + 1) * 512]
                    eng.dma_start(t[:], t_src)
                    xt[c][sg] = t

            # ---- q^T and k^T projections: [DC, S] as 4 tiles [128, S] ----
            # tile t holds local heads 2t (partitions 0:64) and 2t+1 (64:128)
            qT, kT = [], []
            for (w_d, bias_t, out_list, nm) in (
                (wq_d, bq_t, qT, "q"), (wk_d, bk_t, kT, "k"),
            ):
                if nm == "q":
                    wch = wq_pre
                else:
                    wch = []
                    for c in range(NCH):
                        wt = pp.tile([128, DC], bf16, name=f"w{nm}{c}",
                                     tag=f"w{c}")
                        (nc.scalar if c % 2 == 0 else nc.sync).dma_start(
                            wt[:], w_d[c * 128:(c + 1) * 128, :])
                        wch.append(wt)
                for seg in range(4):
                    for t in range(4):
                        if seg == 0:
                            out = pp.tile([128, S], bf16, name=f"{nm}T{t}",
                                          tag=f"{nm}T{t}")
                            out_list.append(out)
                        out = out_list[t]
                        ps = psp.tile([128, 512], f32, name="ps",
                                      tag="big", bufs=2)
                        for c in range(NCH):
                            nc.tensor.matmul(
                                ps[:],
                                wch[c][:, t * 128:(t + 1) * 128],
                                xt[c][seg][:],
                                start=(c == 0), stop=(c == NCH - 1),
                            )
                        nc.vector.tensor_scalar(
                            out[:, seg * 512:(seg + 1) * 512], ps[:],
                            bias_t[:, t:t + 1], 0.0,
                            ALU.add, ALU.max,
                        )

            # ---- v projection into [128, HC, 65] per 128-token block ----
            # cols h,0:64 = relu(x@wv+bv) of local head h; col h,64 = 1.0
            wvch = []
            for c in range(NCH):
                wt = pp.tile([128, DC], bf16, name=f"wv{c}", tag=f"w{c}")
                (nc.scalar if c % 2 == 0 else nc.sync).dma_start(
                    wt[:], wv_d[c * 128:(c + 1) * 128, :])
                wvch.append(wt)
            vav = []
            for st in range(16):
                va = pp.tile([128, HC, 65], bf16, name=f"va{st}",
                             tag=f"va{st}")
                vav.append(va)
                ps = psp.tile([128, 512], f32, name="ps", tag="big", bufs=2)
                for c in range(NCH):
                    nc.tensor.matmul(
                        ps[:],
                        xt[c][st // 4][:, (st % 4) * 128:(st % 4) * 128 + 128],
                        wvch[c][:],
                        start=(c == 0), stop=(c == NCH - 1),
                    )
                nc.vector.tensor_add(ps[:], ps[:], bvb_t[:])
                nc.vector.tensor_scalar(
                    va[:, :, 0:64],
                    ps[:].rearrange("p (h d) -> p h d", h=HC),
                    0.0, None, ALU.max,
                )
                nc.vector.tensor_copy(
                    va[:, :, 64:65],
                    ones_f[:, 0:8].rearrange("p (h o) -> p h o", o=1))

            # ---- wo tiles (needed from o-proj(0) on) ----
            woch = []
            for c in range(NCH):
                wt = pp.tile([128, 512], bf16, name=f"wo{c}", tag=f"wo{c}")
                (nc.gpsimd if c % 2 == 0 else nc.scalar).dma_start(
                    wt[:], wo_d[c * 128:(c + 1) * 128, :])
                woch.append(wt)

            glist = {}

            def attention(j, oj):
                nblk = 4 * j + 4
                for p in range(4):
                    if oj is not None:
                        oproj_qt(oj, p)
                    # ctx accumulators [d 64 + rowsum, q 512], one per head
                    ctxA = cxp.tile([65, 512], f32, name="cx", tag="cx",
                                    bufs=2)
                    ctxB = cxp.tile([65, 512], f32, name="cx", tag="cx",
                                    bufs=2)
                    for kb in range(nblk):
                        dlt = kb * 128 - j * 512
                        qoff = max(dlt, 0)
                        w = 512 - qoff
                        qlo = j * 512 + qoff
                        sc = psp.tile([128, 1024], f32, name="sc",
                                      tag="sc", bufs=2)
                        e2 = ep.tile([128, 1024], bf16, name="e", tag="e",
                                     bufs=5)
                        for hh in (0, 1):
                            plo = hh * 64
                            nc.tensor.matmul(
                                sc[:, hh * 512 + qoff:hh * 512 + 512],
                                kT[p][plo:plo + 64,
                                      kb * 128:(kb + 1) * 128],
                                qT[p][plo:plo + 64, qlo:qlo + w],
                                start=True, stop=True,
                                tile_position=(plo, 0),
                            )
                        if dlt >= 0:
                            # additive causal mask on the diag square
                            nc.vector.tensor_add(
                                sc[:, qoff:qoff + 128],
                                sc[:, qoff:qoff + 128], trineg[:])
                            nc.vector.tensor_add(
                                sc[:, 512 + qoff:512 + qoff + 128],
                                sc[:, 512 + qoff:512 + qoff + 128],
                                trineg[:])
                        if w == 512:
                            nc.scalar.activation(
                                e2[:], sc[:],
                                AF.Exp, bias=expb_t[:], scale=0.125,
                            )
                        else:
                            for hh in (0, 1):
                                nc.scalar.activation(
                                    e2[:, hh * 512 + qoff:hh * 512 + 512],
                                    sc[:, hh * 512 + qoff:hh * 512 + 512],
                                    AF.Exp, bias=expb_t[:], scale=0.125,
                                )
                        for (hh, ctx) in ((0, ctxA), (1, ctxB)):
                            nc.tensor.matmul(
                                ctx[:, qoff:qoff + w],
                                vav[kb][:, 2 * p + hh, :],
                                e2[:, hh * 512 + qoff:hh * 512 + 512],
                                start=(kb == 0), stop=(kb == nblk - 1),
                                skip_group_check=True,
                            )
                    # normalize rows 0:64 by the ones-row sums (row 64) and
                    # assemble the bf16 exchange piece [128 d, 512 q]
                    stgT = sp.tile([128, 512], bf16, name="stgT", tag="stgT",
                                   bufs=3)
                    for (hh, ctx) in ((0, ctxA), (1, ctxB)):
                        rho = sp.tile([1, 512], f32, name="rho", tag="rho",
                                      bufs=4)
                        nc.vector.tensor_copy(rho[:], ctx[64:65, :])
                        rc = sp.tile([1, 512], f32, name="rc", tag="rc",
                                     bufs=4)
                        nc.vector.reciprocal_approx_fast(rc[:], rho[:])
                        rcb = sp.tile([64, 512], f32, name="rcb", tag="rcb",
                                      bufs=4)
                        nc.gpsimd.partition_broadcast(rcb[:], rc[:])
                        nc.vector.tensor_mul(
                            stgT[hh * 64:hh * 64 + 64, :], ctx[0:64, :],
                            rcb[:])
                    idx = j * 4 + p
                    nc.sync.dma_start(cin_d[idx], stgT[:])
                    nc.gpsimd.collective_compute(
                        "AllGather",
                        mybir.AluOpType.bypass,
                        replica_groups=GROUPS,
                        ins=[cin_d[idx].opt()],
                        outs=[cout_d[idx].opt()],
                    )

            def readback(j):
                # deferred so the gpsimd queue never stalls waiting for CC
                # completion mid-attention; must stay on the gpsimd queue
                # (scalar/sync reads of a collective output raced ahead of
                # CC completion on HW)
                for p in range(4):
                    idx = j * 4 + p
                    gA = gp.tile([128, 512], bf16, name=f"g{p}a",
                                 tag=f"g{p}a", bufs=2)
                    gB = gp.tile([128, 512], bf16, name=f"g{p}b",
                                 tag=f"g{p}b", bufs=2)
                    nc.gpsimd.dma_start(gA[:], cout_d[idx, 0:128, :])
                    nc.gpsimd.dma_start(gB[:], cout_d[idx, 128:256, :])
                    glist.setdefault(j, []).append((gA, gB))

            def oproj_qt(j, qt):
                # contraction order [gA0, gB0, gA1, gB1, ...] matches the
                # host's piece-ordered Wo rows, so early pieces' matmuls
                # can run before the last piece's AllGather lands
                chunks = [g for pair in glist[j] for g in pair]
                op = psp.tile([128, 512], f32, name="op", tag="big",
                              bufs=2)
                for c in range(NCH):
                    nc.tensor.matmul(
                        op[:],
                        chunks[c][:, qt * 128:(qt + 1) * 128],
                        woch[c][:],
                        start=(c == 0), stop=(c == NCH - 1),
                    )
                ys = vp.tile([128, 512], f32, name="ys", tag="ys",
                             bufs=3)
                nc.vector.tensor_add(ys[:], op[:], bob_t[:])
                nc.vector.tensor_scalar(ys[:], ys[:], 0.0, None, ALU.max)
                nc.sync.dma_start(
                    y_d[j * 512 + qt * 128:j * 512 + (qt + 1) * 128, :],
                    ys[:])

            # o-proj of chunk j-1 interleaves into attention(j)'s p-loop
            # (one qt group per p) to keep the PE fed during exp waits
            attention(0, None)
            attention(1, None)
            readback(0)
            attention(2, 0)
            readback(1)
            attention(3, 1)
            readback(2)
            for qt in range(4):
                oproj_qt(2, qt)
            readback(3)
            for qt in range(4):
                oproj_qt(3, qt)

    lowp.__exit__(None, None, None)
    nc.compile()
    return nc


def _get_nc():
    if "nc" not in _cache:
        _cache["nc"] = _build()
    return _cache["nc"]


def kernel(x, Wq, bq, Wk, bk, Wv, bv, Wo, bo, trace=False):
    from concourse.bass_utils import run_bass_kernel_spmd

    x = np.asarray(x, np.float32)
    Wq, bq = np.asarray(Wq, np.float32), np.asarray(bq, np.float32)
    Wk, bk = np.asarray(Wk, np.float32), np.asarray(bk, np.float32)
    Wv, bv = np.asarray(Wv, np.float32), np.asarray(bv, np.float32)
    Wo, bo = np.asarray(Wo, np.float32), np.asarray(bo, np.float32)

    nc = _get_nc()
    # Wo rows in piece-arrival order [p0-even, p0-odd, p1-even, p1-odd, ...]
    # (even core = heads 2p,2p+1 -> rows p*128..; odd = heads 8+2p.. ->
    # rows 512+p*128..), matching oproj_qt's chunk order
    worder = []
    for p in range(4):
        worder += list(range(p * 128, (p + 1) * 128))
        worder += list(range(512 + p * 128, 512 + (p + 1) * 128))
    in_maps = []
    for c in range(NCORES):
        b, hh = c // 2, c % 2
        sl = slice(hh * DC, (hh + 1) * DC)
        cols = slice(hh * 512, (hh + 1) * 512)
        in_maps.append({
            "xT": np.ascontiguousarray(x[b].T).astype(ml_dtypes.bfloat16),
            "wq": np.ascontiguousarray(Wq[:, sl]).astype(ml_dtypes.bfloat16),
            "wk": np.ascontiguousarray(Wk[:, sl]).astype(ml_dtypes.bfloat16),
            "wv": np.ascontiguousarray(Wv[:, sl]).astype(ml_dtypes.bfloat16),
            "wo": np.ascontiguousarray(
                Wo[worder][:, cols]).astype(ml_dtypes.bfloat16),
            "bq": np.ascontiguousarray(bq[sl].reshape(4, 128).T),
            "bk": np.ascontiguousarray(bk[sl].reshape(4, 128).T),
            "bvb": np.ascontiguousarray(
                np.broadcast_to(bv[sl], (128, DC))),
            "bob": np.ascontiguousarray(np.broadcast_to(bo[cols], (128, 512))),
        })

    res = run_bass_kernel_spmd(nc, in_maps, core_ids=list(range(NCORES)),
                               trace=trace)
    _cache["last_result"] = res

    y = np.empty((B, S, D), np.float32)
    for c in range(NCORES):
        b, hh = c // 2, c % 2
        y[b, :, hh * 512:(hh + 1) * 512] = res.results[c]["y"]
    return y
